# revision 13
# baseline (speedup 1.0000x reference)
"""Causal self-attention Trainium2 kernel (8 NeuronCores, SPMD).

Problem (hardcoded): B=2, T=2048, C=1024, H=16 heads, d=64.
  qkv = x @ W_qkv ; per-head causal softmax attention ; out @ W_proj.

Sharding: core m (0..7) handles batch b = m//4 and head group g = m%4
(heads 4g..4g+3). Each core computes q/k/v for its 4 heads (256 of the
3072 W_qkv columns), full TxT causal attention for those heads, and a
partial projection y_m = att_m @ W_proj[256g:256g+256, :].  The host
sums the 4 partials per batch (row-split tensor parallel reduce).

Device kernel layout notes (scores kept TRANSPOSED: [key j, query i]):
  - x is fed pre-transposed per batch: xT [C, T] (fp16).
  - qkv^T computed as matmul(lhsT=W block, rhs=xT block): q^T/k^T land
    in [head-ch, T] layout, exactly what QK^T needs (contract over d).
  - v is computed in natural [T, ch] layout (lhsT=xT block, rhs=Wv) and
    stored with an extra ones column per head, so the AV matmul also
    yields the softmax denominators (replicated on the opposite 64
    partitions of the data).
  - scores^T tiles [128 j, 512 i]: only j-blocks <= diagonal are
    computed (causal skip ~2x FLOPs); diagonal tiles are masked AFTER
    exp by zeroing with gpsimd.affine_select (keep iff j <= i).
  - engine split: PE matmuls, ACT exp only, DVE everything elementwise
    (copies/casts/recip/normalize), GPSIMD causal masks, DMA shifts.
  - the two heads of a pair use PE row groups (0,0)/(64,0): QK matmuls
    are issued h2-alternating so consecutive MMs land in disjoint row
    groups and execute concurrently (2-way PE tiling).
  - software pipeline: the ACT-bound attention group loop pulls ~1.7us
    "pieces" of next-chunk qkv / prev-chunk projection PE work from a
    queue between groups, keeping the PE dense while ACT exps.
"""

from collections import deque

import numpy as np

import concourse.bass as bass
import concourse.mybir as mybir
import concourse.tile as tile
from concourse import bacc

FP32 = mybir.dt.float32
FP16 = mybir.dt.float16
AF = mybir.ActivationFunctionType
ALU = mybir.AluOpType

B, T_FULL, C_FULL, H_FULL, D_HEAD = 2, 2048, 1024, 16, 64
N_CORES = 8


def build_nc(T=T_FULL, C=C_FULL, HD=4, D=D_HEAD, n_cores=N_CORES):
    """Build the per-core Bass program. HD = heads per core."""
    CD = HD * D              # device head channels (256)
    CB = C // 128            # contraction blocks over x/W channels
    ICH = 512                # query-chunk width
    S = ICH // 128           # j-blocks per query chunk on the diagonal
    TC = 512                 # token chunk in qkv phase
    NTC = T // TC
    NTB = T // 128           # 128-token blocks (= key blocks)
    NPAIR = HD // 2
    JQK = CD // 128          # q (and k) 128-wide column blocks
    assert JQK == NPAIR and T % ICH == 0 and C % 512 == 0
    softmax_scale = 1.0 / float(np.sqrt(D))

    nc = bacc.Bacc(
        "TRN2", target_bir_lowering=False, debug=False, num_devices=n_cores
    )
    xT = nc.dram_tensor("xT", [C, T], FP16, kind="ExternalInput").ap()
    wqkv = nc.dram_tensor("wqkv", [C, 3 * CD], FP16, kind="ExternalInput").ap()
    wp = nc.dram_tensor("wp", [CD, C], FP16, kind="ExternalInput").ap()
    y = nc.dram_tensor("y", [T, C], FP16, kind="ExternalOutput").ap()

    with tile.TileContext(nc) as tc:
        with (
            tc.tile_pool(name="consts", bufs=1) as consts,
            tc.tile_pool(name="xt", bufs=2 * CB) as xt_pool,
            tc.tile_pool(name="ew", bufs=4) as ew_pool,
            tc.tile_pool(name="small", bufs=6) as small_pool,
            tc.tile_pool(name="ysb", bufs=4) as ysb_pool,
            tc.tile_pool(name="pssc", bufs=2, space="PSUM") as ps_sc,
            tc.tile_pool(name="psqk", bufs=2, space="PSUM") as ps_qk,
            tc.tile_pool(name="psav", bufs=2, space="PSUM") as ps_av,
        ):
            # ---- startup: interleave chunk-0 x and per-cb weight DMAs so
            # the first qkv accumulation steps start after ~0.4MB, not
            # after the full 2.5MB preload.
            xts0 = []
            w_sb = []
            for cb in range(CB):
                xt_t = xt_pool.tile([128, TC], FP16, tag="xt")
                nc.sync.dma_start(xt_t, xT[128 * cb : 128 * (cb + 1), 0:TC])
                xts0.append(xt_t)
                w_cb = consts.tile([128, 3 * CD], FP16, name=f"w{cb}")
                nc.sync.dma_start(
                    w_cb, wqkv[128 * cb : 128 * (cb + 1), :]
                )
                w_sb.append(w_cb)

            # PE warmup: dense stream of tiny matmuls (no data deps) so the
            # HAM clock gate reaches K=8/8 while the input DMAs stream.
            # ~9us of dummy matmuls: bridges the input-DMA wait with dense
            # PE activity so the HAM clock gate is at K=8/8 (2.4 GHz) when
            # the real qkv matmuls start (HAM needs ~3.4us sustained busy,
            # and an idle gap >3.4us would re-throttle).
            warm_z = consts.tile([128, 64], FP16)
            nc.gpsimd.memset(warm_z, 0.0)
            warm_ps = ps_qk.tile([64, 64], FP32, tag="qk")
            for _ in range(184):
                nc.tensor.matmul(warm_ps, warm_z, warm_z, start=True, stop=True)

            # ---- resident tensors ----
            qT = consts.tile([128, NPAIR, T], FP16)
            kT = consts.tile([128, NPAIR, T], FP16)
            # v with 64 replicated ones-columns per head: the AV matmul
            # (M=128, free) then emits data on one partition half and the
            # softmax denominator replicated across the other half --
            # denominator broadcast comes for free.  Even heads (h2=0):
            # [ones | v] -> av rows 0-63 = sums, 64-127 = data.  Odd
            # heads: [v | ones] -> av rows 0-63 = data, 64-127 = sums.
            vS = consts.tile([128, NTB, NPAIR, 2, 2, D], FP16)
            nc.vector.memset(vS[:, :, :, 0, 0, :], 1.0)
            nc.vector.memset(vS[:, :, :, 1, 1, :], 1.0)
            attT = consts.tile([128, NPAIR, T], FP16)
            wp_sb = consts.tile([128, CD // 128, C], FP16)

            # ================= piece queues =================
            # urgent: next-chunk qkv pieces (must fully drain before that
            # chunk's attention). lazy: projection pieces (no deadline).
            urgent_q = deque()
            lazy_q = deque()

            def emit_piece():
                if urgent_q:
                    urgent_q.popleft()()
                elif lazy_q:
                    lazy_q.popleft()()

            def drain_urgent():
                while urgent_q:
                    urgent_q.popleft()()

            # ---- qkv pieces for one token chunk ----
            def queue_qkv(t, xts):
                def qk_piece(qk, jb, t=t, xts=xts):
                    p2 = ps_qk.tile([128, TC], FP32, tag="qk")
                    co = CD * qk + 128 * jb
                    for cb in range(CB):
                        nc.tensor.matmul(
                            p2,
                            w_sb[cb][:, co : co + 128],
                            xts[cb],
                            start=(cb == 0),
                            stop=(cb == CB - 1),
                        )
                    dst = qT if qk == 0 else kT
                    with nc.allow_low_precision("qk fp16"):
                        nc.vector.tensor_copy(
                            out=dst[:, jb, TC * t : TC * (t + 1)], in_=p2
                        )

                def v_piece(tb, t=t, xts=xts):
                    tb_g = t * (TC // 128) + tb
                    pv = ps_qk.tile([128, CD], FP32, tag="qk")
                    for cb in range(CB):
                        nc.tensor.matmul(
                            pv,
                            xts[cb][:, 128 * tb : 128 * (tb + 1)],
                            w_sb[cb][:, 2 * CD : 3 * CD],
                            start=(cb == 0),
                            stop=(cb == CB - 1),
                        )
                    pv4 = pv.rearrange("p (pr h d) -> p pr h d", h=2, d=D)
                    with nc.allow_low_precision("v fp16"):
                        nc.vector.tensor_copy(
                            out=vS[:, tb_g, :, 0, 1, :], in_=pv4[:, :, 0, :]
                        )
                        nc.vector.tensor_copy(
                            out=vS[:, tb_g, :, 1, 0, :], in_=pv4[:, :, 1, :]
                        )

                for qk in range(2):
                    for jb in range(JQK):
                        urgent_q.append(
                            lambda qk=qk, jb=jb: qk_piece(qk, jb)
                        )
                for tb in range(TC // 128):
                    urgent_q.append(lambda tb=tb: v_piece(tb))

            def prefetch_x(t):
                xts = []
                for cb in range(CB):
                    xt_t = xt_pool.tile([128, TC], FP16, tag="xt")
                    nc.sync.dma_start(
                        xt_t, xT[128 * cb : 128 * (cb + 1), TC * t : TC * (t + 1)]
                    )
                    xts.append(xt_t)
                return xts

            # ---- projection pieces for one token chunk ----
            def queue_proj(t):
                def proj_piece(tb, nck):
                    py = ps_qk.tile([128, 512], FP32, tag="qk")
                    for p in range(NPAIR):
                        nc.tensor.matmul(
                            py,
                            attT[:, p, 128 * tb : 128 * (tb + 1)],
                            wp_sb[:, p, 512 * nck : 512 * (nck + 1)],
                            start=(p == 0),
                            stop=(p == NPAIR - 1),
                        )
                    y_sb = ysb_pool.tile([128, 512], FP16, tag="ysb")
                    with nc.allow_low_precision("y fp16"):
                        nc.vector.tensor_copy(out=y_sb, in_=py)
                    nc.sync.dma_start(
                        y[128 * tb : 128 * (tb + 1), 512 * nck : 512 * (nck + 1)],
                        y_sb,
                    )

                for tb in range(t * (TC // 128), (t + 1) * (TC // 128)):
                    for nck in range(C // 512):
                        lazy_q.append(
                            lambda tb=tb, nck=nck: proj_piece(tb, nck)
                        )

            # ================= attention =================
            # Softmax tail per (p, ic): approx-reciprocal of the replicated
            # sums halves straight from PSUM, fp16 cast of the data halves,
            # DMA partition-shift into attT, one fused normalize multiply.
            # Tails are deferred one iteration (pending) so their DVE work
            # overlaps the next group's QK/exp instead of head-of-line
            # blocking the DVE queue.
            def emit_tail(p, ic, av):
                ics = slice(ICH * ic, ICH * (ic + 1))
                s32 = small_pool.tile([128, ICH], FP32, tag="sums")
                r16 = small_pool.tile([128, ICH], FP16, tag="recip")
                t16 = small_pool.tile([128, ICH], FP16, tag="tmp")
                # h2=0: rows 0-63 = sums, 64-127 = data; h2=1 mirrored.
                # Sums packed into one full-partition SBUF fp32 tile so a
                # single reciprocal_approx_fast covers both heads (custom
                # DVE op: SBUF base-0 operands only).
                nc.vector.tensor_copy(out=s32[0:64, :], in_=av[0][0:64, :])
                nc.vector.tensor_copy(out=s32[64:128, :], in_=av[1][64:128, :])
                with nc.allow_low_precision("softmax raw fp16"):
                    nc.vector.tensor_copy(
                        out=t16[64:128, :], in_=av[0][64:128, :]
                    )
                    nc.vector.tensor_copy(out=t16[0:64, :], in_=av[1][0:64, :])
                nc.vector.reciprocal_approx_fast(s32, s32)
                with nc.allow_low_precision("softmax recip fp16"):
                    nc.vector.tensor_copy(out=r16, in_=s32)
                nc.gpsimd.dma_start(attT[0:64, p, ics], t16[64:128, :])
                nc.gpsimd.dma_start(attT[64:128, p, ics], t16[0:64, :])
                dst = attT[:, p, ics]
                with nc.allow_low_precision("softmax norm fp16"):
                    nc.vector.tensor_tensor(dst, dst, r16, ALU.mult)

            pending = None

            def attention_ic(p, ic):
                nonlocal pending
                J = S * (ic + 1)          # kept key blocks for this chunk
                n_grp = (J + 1) // 2
                av = [
                    ps_av.tile([128, ICH], FP32, tag="av", name=f"av{h2}")
                    for h2 in range(2)
                ]

                def q0_of(jb, ic=ic):
                    # first causally-live query (chunk-local) of j-block jb
                    s = jb - S * ic
                    return 128 * s if s > 0 else 0

                def qk_group(g, p=p, ic=ic, J=J):
                    """scores^T + exp + causal mask for 2 j-blocks, both
                    heads of the pair; QK matmuls issued h2-alternating so
                    they pair up in disjoint PE row groups.  Diagonal
                    j-blocks only compute queries >= the block's first live
                    query; the 128-wide triangle is masked after exp."""
                    jbs = [jb for jb in (2 * g, 2 * g + 1) if jb < J]
                    diag = any(jb - S * ic >= 0 for jb in jbs)
                    scs = [
                        ps_sc.tile(
                            [128, len(jbs) * ICH], FP32, tag="sc",
                            name=f"sc{h2}",
                        )
                        for h2 in range(2)
                    ]
                    for i_s, jb in enumerate(jbs):
                        q0 = q0_of(jb)
                        for h2 in range(2):
                            po = 64 * h2
                            nc.tensor.matmul(
                                scs[h2][:, ICH * i_s + q0 : ICH * (i_s + 1)],
                                kT[po : po + 64, p, 128 * jb : 128 * (jb + 1)],
                                qT[po : po + 64, p,
                                   ICH * ic + q0 : ICH * (ic + 1)],
                                start=True,
                                stop=True,
                                tile_position=(po, 0),
                            )
                    ews = []
                    for h2 in range(2):
                        ew = ew_pool.tile([128, len(jbs), ICH], FP16, tag="ew")
                        if not diag:
                            nc.scalar.activation(
                                ew.rearrange("p a b -> p (a b)"),
                                scs[h2],
                                AF.Exp,
                                scale=softmax_scale,
                            )
                        else:
                            for i_s, jb in enumerate(jbs):
                                q0 = q0_of(jb)
                                nc.scalar.activation(
                                    ew[:, i_s, q0:],
                                    scs[h2][:, ICH * i_s + q0 : ICH * (i_s + 1)],
                                    AF.Exp,
                                    scale=softmax_scale,
                                )
                                # triangle: zero where key > query
                                nc.gpsimd.affine_select(
                                    out=ew[:, i_s, q0 : q0 + 128],
                                    in_=ew[:, i_s, q0 : q0 + 128],
                                    compare_op=ALU.is_ge,
                                    fill=0.0,
                                    base=0,
                                    pattern=[[1, 128]],
                                    channel_multiplier=-1,
                                )
                        ews.append((ew, jbs))
                    return ews

                def av_group(ews, p=p, J=J):
                    for h2 in range(2):
                        ew, jbs = ews[h2]
                        for i_s, jb in enumerate(jbs):
                            q0 = q0_of(jb)
                            nc.tensor.matmul(
                                av[h2][:, q0:],
                                vS[:, jb, p, h2].rearrange("p a b -> p (a b)"),
                                ew[:, i_s, q0:],
                                start=(jb == 0),
                                stop=(jb == J - 1),
                            )

                # 1-group software pipeline: QK(g+1) before AV(g); one PE
                # piece from the queue per group; the previous (p, ic) tail
                # goes after the next QK group so DVE never stalls the PE.
                prev = qk_group(0)
                for g in range(1, n_grp):
                    cur = qk_group(g)
                    emit_piece()
                    if pending is not None:
                        emit_tail(*pending)
                        pending = None
                    av_group(prev)
                    prev = cur
                if pending is not None:
                    emit_tail(*pending)
                    pending = None
                emit_piece()
                av_group(prev)
                pending = (p, ic, av)

            # ================= main schedule =================
            # chunk 0 qkv emitted directly; later chunks' qkv and earlier
            # chunks' projection drip in as pieces between attention groups.
            queue_qkv(0, xts0)
            drain_urgent()
            nc.sync.dma_start(wp_sb, wp.rearrange("(po pi) f -> pi po f", pi=128))
            for t in range(NTC):
                if t + 1 < NTC:
                    queue_qkv(t + 1, prefetch_x(t + 1))
                attention_ic(0, t)
                if t - 1 >= 0:
                    queue_proj(t - 1)   # after att(0,t): tails t-1 flushed
                attention_ic(1, t)
                drain_urgent()
            while lazy_q:
                lazy_q.popleft()()
            # Final-chunk projection, pair-split: the p=0 partial matmuls
            # only need pair 0's attT (ready before the last tail), so they
            # run while the final tail's DVE/DMA chain drains; p=1 + the y
            # copy/DMA follow the tail.  PSUM for the partials is scrounged
            # from the sc/qk/av pools as attention releases them.
            tl = NTC - 1
            blocks = [
                (tb, nck)
                for tb in range(tl * (TC // 128), (tl + 1) * (TC // 128))
                for nck in range(C // 512)
            ]
            slots = []
            for i in range(min(4, len(blocks)) // 2):
                big = ps_sc.tile([128, 1024], FP32, tag="sc", name=f"pyb{i}")
                slots += [(big, 0), (big, 512)]
            for i in range(min(2, len(blocks) - len(slots))):
                tqk = ps_qk.tile([128, 512], FP32, tag="qk", name=f"pyq{i}")
                slots.append((tqk, 0))

            def proj_A(i):
                t_, off = slots[i]
                tb, nck = blocks[i]
                nc.tensor.matmul(
                    t_[:, off : off + 512],
                    attT[:, 0, 128 * tb : 128 * (tb + 1)],
                    wp_sb[:, 0, 512 * nck : 512 * (nck + 1)],
                    start=True,
                    stop=False,
                )

            def proj_B(i):
                t_, off = slots[i]
                tb, nck = blocks[i]
                nc.tensor.matmul(
                    t_[:, off : off + 512],
                    attT[:, 1, 128 * tb : 128 * (tb + 1)],
                    wp_sb[:, 1, 512 * nck : 512 * (nck + 1)],
                    start=False,
                    stop=True,
                )
                y_sb = ysb_pool.tile([128, 512], FP16, tag="ysb")
                nc.scalar.copy(out=y_sb, in_=t_[:, off : off + 512])
                eng = nc.sync if i % 2 == 0 else nc.gpsimd
                eng.dma_start(
                    y[128 * tb : 128 * (tb + 1), 512 * nck : 512 * (nck + 1)],
                    y_sb,
                )

            n_pre = len(slots)
            for i in range(n_pre):
                proj_A(i)
            emit_tail(*pending)
            for i in range(n_pre, len(blocks)):
                tav = ps_av.tile([128, 512], FP32, tag="av", name=f"pya{i}")
                slots.append((tav, 0))
                proj_A(i)
            for i in range(len(blocks)):
                proj_B(i)
    nc.compile()
    return nc


def make_in_maps(x, W_qkv, W_proj, n_cores=N_CORES):
    """Host-side sharding: per-core fp16 inputs."""
    Bv, T, C = x.shape
    groups = n_cores // Bv
    CD = C // groups
    in_maps = []
    xT_b = [np.ascontiguousarray(x[b].T).astype(np.float16) for b in range(Bv)]
    for m in range(n_cores):
        b, g = m // groups, m % groups
        cols = slice(CD * g, CD * (g + 1))
        wqkv_dev = np.ascontiguousarray(
            np.concatenate(
                [W_qkv[:, 0:C][:, cols], W_qkv[:, C : 2 * C][:, cols],
                 W_qkv[:, 2 * C : 3 * C][:, cols]],
                axis=1,
            ).astype(np.float16)
        )
        wp_dev = np.ascontiguousarray(W_proj[cols, :]).astype(np.float16)
        in_maps.append({"xT": xT_b[b], "wqkv": wqkv_dev, "wp": wp_dev})
    return in_maps


_cache = {}


def _get_nc():
    if "nc" not in _cache:
        _cache["nc"] = build_nc()
    return _cache["nc"]


def run(x, W_qkv, W_proj, trace=False):
    """Run on hardware; returns (y_full, BassKernelResults)."""
    from concourse.bass_utils import run_bass_kernel_spmd

    nc = _get_nc()
    in_maps = make_in_maps(x, W_qkv, W_proj)
    res = run_bass_kernel_spmd(
        nc, in_maps, core_ids=list(range(N_CORES)), trace=trace
    )
    Bv, T, C = x.shape
    groups = N_CORES // Bv
    y_full = np.zeros((Bv, T, C), dtype=np.float32)
    for m in range(N_CORES):
        y_full[m // groups] += res.results[m]["y"].astype(np.float32)
    return y_full, res


def kernel(x, W_qkv, W_proj):
    y, _ = run(
        np.asarray(x, dtype=np.float32),
        np.asarray(W_qkv, dtype=np.float32),
        np.asarray(W_proj, dtype=np.float32),
    )
    return y


# revision 14
# speedup vs baseline: 1.1999x; 1.1999x over previous
"""Causal self-attention Trainium2 kernel (8 NeuronCores, SPMD).

Problem (hardcoded): B=2, T=2048, C=1024, H=16 heads, d=64.
  qkv = x @ W_qkv ; per-head causal softmax attention ; out @ W_proj.

Sharding: core m (0..7) handles batch b = m//4 and head group g = m%4
(heads 4g..4g+3). Each core computes q/k/v for its 4 heads (256 of the
3072 W_qkv columns), full TxT causal attention for those heads, and a
partial projection y_m = att_m @ W_proj[256g:256g+256, :].  The host
sums the 4 partials per batch (row-split tensor parallel reduce).

Device kernel layout notes (scores kept TRANSPOSED: [key j, query i]):
  - x is fed pre-transposed per batch: xT [C, T] (fp16).
  - qkv^T computed as matmul(lhsT=W block, rhs=xT block): q^T/k^T land
    in [head-ch, T] layout, exactly what QK^T needs (contract over d).
  - v is computed in natural [T, ch] layout (lhsT=xT block, rhs=Wv) and
    stored with an extra ones column per head, so the AV matmul also
    yields the softmax denominators (replicated on the opposite 64
    partitions of the data).
  - scores^T tiles [128 j, 512 i]: only j-blocks <= diagonal are
    computed (causal skip ~2x FLOPs); diagonal tiles are masked AFTER
    exp by zeroing with gpsimd.affine_select (keep iff j <= i).
  - engine split: PE matmuls, ACT exp only, DVE everything elementwise
    (copies/casts/recip/normalize), GPSIMD causal masks, DMA shifts.
  - the two heads of a pair use PE row groups (0,0)/(64,0): QK matmuls
    are issued h2-alternating so consecutive MMs land in disjoint row
    groups and execute concurrently (2-way PE tiling).
  - software pipeline: the ACT-bound attention group loop pulls ~1.7us
    "pieces" of next-chunk qkv / prev-chunk projection PE work from a
    queue between groups, keeping the PE dense while ACT exps.
"""

from collections import deque

import numpy as np

import concourse.bass as bass
import concourse.mybir as mybir
import concourse.tile as tile
from concourse import bacc

FP32 = mybir.dt.float32
FP16 = mybir.dt.float16
AF = mybir.ActivationFunctionType
ALU = mybir.AluOpType

B, T_FULL, C_FULL, H_FULL, D_HEAD = 2, 2048, 1024, 16, 64
N_CORES = 8


def build_nc(T=T_FULL, C=C_FULL, HD=4, D=D_HEAD, n_cores=N_CORES):
    """Build the per-core Bass program. HD = heads per core."""
    CD = HD * D              # device head channels (256)
    CB = C // 128            # contraction blocks over x/W channels
    ICH = 512                # query-chunk width
    S = ICH // 128           # j-blocks per query chunk on the diagonal
    TC = 512                 # token chunk in qkv phase
    NTC = T // TC
    NTB = T // 128           # 128-token blocks (= key blocks)
    NPAIR = HD // 2
    JQK = CD // 128          # q (and k) 128-wide column blocks
    assert JQK == NPAIR and T % ICH == 0 and C % 512 == 0
    softmax_scale = 1.0 / float(np.sqrt(D))

    nc = bacc.Bacc(
        "TRN2", target_bir_lowering=False, debug=False, num_devices=n_cores
    )
    xT = nc.dram_tensor("xT", [C, T], FP16, kind="ExternalInput").ap()
    wqkv = nc.dram_tensor("wqkv", [C, 3 * CD], FP16, kind="ExternalInput").ap()
    wp = nc.dram_tensor("wp", [CD, C], FP16, kind="ExternalInput").ap()
    y = nc.dram_tensor("y", [T, C], FP16, kind="ExternalOutput").ap()

    with tile.TileContext(nc) as tc:
        with (
            tc.tile_pool(name="consts", bufs=1) as consts,
            tc.tile_pool(name="xt", bufs=2 * CB) as xt_pool,
            tc.tile_pool(name="ew", bufs=4) as ew_pool,
            tc.tile_pool(name="small", bufs=6) as small_pool,
            tc.tile_pool(name="ysb", bufs=4) as ysb_pool,
            tc.tile_pool(name="pssc", bufs=2, space="PSUM") as ps_sc,
            tc.tile_pool(name="psqk", bufs=2, space="PSUM") as ps_qk,
            tc.tile_pool(name="psav", bufs=2, space="PSUM") as ps_av,
        ):
            # ---- startup: interleave chunk-0 x and per-cb weight DMAs so
            # the first qkv accumulation steps start after ~0.4MB, not
            # after the full 2.5MB preload.
            xts0 = []
            w_sb = []
            for cb in range(CB):
                xt_t = xt_pool.tile([128, TC], FP16, tag="xt")
                nc.sync.dma_start(xt_t, xT[128 * cb : 128 * (cb + 1), 0:TC])
                xts0.append(xt_t)
                w_cb = consts.tile([128, 3 * CD], FP16, name=f"w{cb}")
                nc.sync.dma_start(
                    w_cb, wqkv[128 * cb : 128 * (cb + 1), :]
                )
                w_sb.append(w_cb)

            # PE warmup: dense stream of tiny matmuls (no data deps) so the
            # HAM clock gate reaches K=8/8 while the input DMAs stream.
            # ~9us of dummy matmuls: bridges the input-DMA wait with dense
            # PE activity so the HAM clock gate is at K=8/8 (2.4 GHz) when
            # the real qkv matmuls start (HAM needs ~3.4us sustained busy,
            # and an idle gap >3.4us would re-throttle).
            warm_z = consts.tile([128, 64], FP16)
            nc.gpsimd.memset(warm_z, 0.0)
            warm_ps = ps_qk.tile([64, 64], FP32, tag="qk")
            for _ in range(184):
                nc.tensor.matmul(warm_ps, warm_z, warm_z, start=True, stop=True)

            # ---- resident tensors ----
            qT = consts.tile([128, NPAIR, T], FP16)
            kT = consts.tile([128, NPAIR, T], FP16)
            # v with 64 replicated ones-columns per head: the AV matmul
            # (M=128, free) then emits data on one partition half and the
            # softmax denominator replicated across the other half --
            # denominator broadcast comes for free.  Even heads (h2=0):
            # [ones | v] -> av rows 0-63 = sums, 64-127 = data.  Odd
            # heads: [v | ones] -> av rows 0-63 = data, 64-127 = sums.
            vS = consts.tile([128, NTB, NPAIR, 2, 2, D], FP16)
            nc.vector.memset(vS[:, :, :, 0, 0, :], 1.0)
            nc.vector.memset(vS[:, :, :, 1, 1, :], 1.0)
            attT = consts.tile([128, NPAIR, T], FP16)
            wp_sb = consts.tile([128, CD // 128, C], FP16)

            # ================= piece queues =================
            # urgent: next-chunk qkv pieces (must fully drain before that
            # chunk's attention). lazy: projection pieces (no deadline).
            urgent_q = deque()
            lazy_q = deque()

            def emit_piece():
                if urgent_q:
                    urgent_q.popleft()()
                elif lazy_q:
                    lazy_q.popleft()()

            def drain_urgent():
                while urgent_q:
                    urgent_q.popleft()()

            # ---- qkv pieces for one token chunk ----
            def queue_qkv(t, xts):
                def qk_piece(qk, jb, t=t, xts=xts):
                    p2 = ps_qk.tile([128, TC], FP32, tag="qk")
                    co = CD * qk + 128 * jb
                    for cb in range(CB):
                        nc.tensor.matmul(
                            p2,
                            w_sb[cb][:, co : co + 128],
                            xts[cb],
                            start=(cb == 0),
                            stop=(cb == CB - 1),
                        )
                    dst = qT if qk == 0 else kT
                    with nc.allow_low_precision("qk fp16"):
                        nc.vector.tensor_copy(
                            out=dst[:, jb, TC * t : TC * (t + 1)], in_=p2
                        )

                def v_piece(tb, t=t, xts=xts):
                    tb_g = t * (TC // 128) + tb
                    pv = ps_qk.tile([128, CD], FP32, tag="qk")
                    for cb in range(CB):
                        nc.tensor.matmul(
                            pv,
                            xts[cb][:, 128 * tb : 128 * (tb + 1)],
                            w_sb[cb][:, 2 * CD : 3 * CD],
                            start=(cb == 0),
                            stop=(cb == CB - 1),
                        )
                    pv4 = pv.rearrange("p (pr h d) -> p pr h d", h=2, d=D)
                    with nc.allow_low_precision("v fp16"):
                        nc.vector.tensor_copy(
                            out=vS[:, tb_g, :, 0, 1, :], in_=pv4[:, :, 0, :]
                        )
                        nc.vector.tensor_copy(
                            out=vS[:, tb_g, :, 1, 0, :], in_=pv4[:, :, 1, :]
                        )

                for qk in range(2):
                    for jb in range(JQK):
                        urgent_q.append(
                            lambda qk=qk, jb=jb: qk_piece(qk, jb)
                        )
                for tb in range(TC // 128):
                    urgent_q.append(lambda tb=tb: v_piece(tb))

            def prefetch_x(t):
                xts = []
                for cb in range(CB):
                    xt_t = xt_pool.tile([128, TC], FP16, tag="xt")
                    nc.sync.dma_start(
                        xt_t, xT[128 * cb : 128 * (cb + 1), TC * t : TC * (t + 1)]
                    )
                    xts.append(xt_t)
                return xts

            # ---- projection pieces for one token chunk ----
            def queue_proj(t):
                def proj_piece(tb, nck):
                    py = ps_qk.tile([128, 512], FP32, tag="qk")
                    for p in range(NPAIR):
                        nc.tensor.matmul(
                            py,
                            attT[:, p, 128 * tb : 128 * (tb + 1)],
                            wp_sb[:, p, 512 * nck : 512 * (nck + 1)],
                            start=(p == 0),
                            stop=(p == NPAIR - 1),
                        )
                    y_sb = ysb_pool.tile([128, 512], FP16, tag="ysb")
                    with nc.allow_low_precision("y fp16"):
                        nc.vector.tensor_copy(out=y_sb, in_=py)
                    nc.sync.dma_start(
                        y[128 * tb : 128 * (tb + 1), 512 * nck : 512 * (nck + 1)],
                        y_sb,
                    )

                for tb in range(t * (TC // 128), (t + 1) * (TC // 128)):
                    for nck in range(C // 512):
                        lazy_q.append(
                            lambda tb=tb, nck=nck: proj_piece(tb, nck)
                        )

            # ================= attention =================
            # Softmax tail per (p, ic): approx-reciprocal of the replicated
            # sums halves straight from PSUM, fp16 cast of the data halves,
            # DMA partition-shift into attT, one fused normalize multiply.
            # Tails are deferred one iteration (pending) so their DVE work
            # overlaps the next group's QK/exp instead of head-of-line
            # blocking the DVE queue.
            def emit_tail(p, ic, av):
                ics = slice(ICH * ic, ICH * (ic + 1))
                s32 = small_pool.tile([128, ICH], FP32, tag="sums")
                r16 = small_pool.tile([128, ICH], FP16, tag="recip")
                t16 = small_pool.tile([128, ICH], FP16, tag="tmp")
                # h2=0: rows 0-63 = sums, 64-127 = data; h2=1 mirrored.
                # Sums packed into one full-partition SBUF fp32 tile so a
                # single reciprocal_approx_fast covers both heads (custom
                # DVE op: SBUF base-0 operands only).
                nc.vector.tensor_copy(out=s32[0:64, :], in_=av[0][0:64, :])
                nc.vector.tensor_copy(out=s32[64:128, :], in_=av[1][64:128, :])
                with nc.allow_low_precision("softmax raw fp16"):
                    nc.vector.tensor_copy(
                        out=t16[64:128, :], in_=av[0][64:128, :]
                    )
                    nc.vector.tensor_copy(out=t16[0:64, :], in_=av[1][0:64, :])
                nc.vector.reciprocal_approx_fast(s32, s32)
                with nc.allow_low_precision("softmax recip fp16"):
                    nc.vector.tensor_copy(out=r16, in_=s32)
                nc.sync.dma_start(attT[0:64, p, ics], t16[64:128, :])
                nc.sync.dma_start(attT[64:128, p, ics], t16[0:64, :])
                dst = attT[:, p, ics]
                with nc.allow_low_precision("softmax norm fp16"):
                    nc.vector.tensor_tensor(dst, dst, r16, ALU.mult)

            pending = None

            def attention_ic(p, ic):
                nonlocal pending
                J = S * (ic + 1)          # kept key blocks for this chunk
                n_grp = (J + 1) // 2
                av = [
                    ps_av.tile([128, ICH], FP32, tag="av", name=f"av{h2}")
                    for h2 in range(2)
                ]

                def q0_of(jb, ic=ic):
                    # first causally-live query (chunk-local) of j-block jb
                    s = jb - S * ic
                    return 128 * s if s > 0 else 0

                def qk_group(g, p=p, ic=ic, J=J):
                    """scores^T + exp + causal mask for 2 j-blocks, both
                    heads of the pair; QK matmuls issued h2-alternating so
                    they pair up in disjoint PE row groups.  Diagonal
                    j-blocks only compute queries >= the block's first live
                    query; the 128-wide triangle is masked after exp."""
                    jbs = [jb for jb in (2 * g, 2 * g + 1) if jb < J]
                    diag = any(jb - S * ic >= 0 for jb in jbs)
                    scs = [
                        ps_sc.tile(
                            [128, len(jbs) * ICH], FP32, tag="sc",
                            name=f"sc{h2}",
                        )
                        for h2 in range(2)
                    ]
                    for i_s, jb in enumerate(jbs):
                        q0 = q0_of(jb)
                        for h2 in range(2):
                            po = 64 * h2
                            nc.tensor.matmul(
                                scs[h2][:, ICH * i_s + q0 : ICH * (i_s + 1)],
                                kT[po : po + 64, p, 128 * jb : 128 * (jb + 1)],
                                qT[po : po + 64, p,
                                   ICH * ic + q0 : ICH * (ic + 1)],
                                start=True,
                                stop=True,
                                tile_position=(po, 0),
                            )
                    ews = []
                    for h2 in range(2):
                        ew = ew_pool.tile([128, len(jbs), ICH], FP16, tag="ew")
                        if not diag:
                            nc.scalar.activation(
                                ew.rearrange("p a b -> p (a b)"),
                                scs[h2],
                                AF.Exp,
                                scale=softmax_scale,
                            )
                        else:
                            for i_s, jb in enumerate(jbs):
                                q0 = q0_of(jb)
                                nc.scalar.activation(
                                    ew[:, i_s, q0:],
                                    scs[h2][:, ICH * i_s + q0 : ICH * (i_s + 1)],
                                    AF.Exp,
                                    scale=softmax_scale,
                                )
                                # triangle: zero where key > query
                                nc.gpsimd.affine_select(
                                    out=ew[:, i_s, q0 : q0 + 128],
                                    in_=ew[:, i_s, q0 : q0 + 128],
                                    compare_op=ALU.is_ge,
                                    fill=0.0,
                                    base=0,
                                    pattern=[[1, 128]],
                                    channel_multiplier=-1,
                                )
                        ews.append((ew, jbs))
                    return ews

                def av_group(ews, p=p, J=J):
                    for h2 in range(2):
                        ew, jbs = ews[h2]
                        for i_s, jb in enumerate(jbs):
                            q0 = q0_of(jb)
                            nc.tensor.matmul(
                                av[h2][:, q0:],
                                vS[:, jb, p, h2].rearrange("p a b -> p (a b)"),
                                ew[:, i_s, q0:],
                                start=(jb == 0),
                                stop=(jb == J - 1),
                            )

                # 1-group software pipeline: QK(g+1) before AV(g); one PE
                # piece from the queue per group; the previous (p, ic) tail
                # goes after the next QK group so DVE never stalls the PE.
                prev = qk_group(0)
                for g in range(1, n_grp):
                    cur = qk_group(g)
                    emit_piece()
                    if pending is not None:
                        emit_tail(*pending)
                        pending = None
                    av_group(prev)
                    prev = cur
                if pending is not None:
                    emit_tail(*pending)
                    pending = None
                emit_piece()
                av_group(prev)
                pending = (p, ic, av)

            # ================= main schedule =================
            # chunk 0 qkv emitted directly; later chunks' qkv and earlier
            # chunks' projection drip in as pieces between attention groups.
            queue_qkv(0, xts0)
            drain_urgent()
            nc.sync.dma_start(wp_sb, wp.rearrange("(po pi) f -> pi po f", pi=128))
            for t in range(NTC):
                if t + 1 < NTC:
                    queue_qkv(t + 1, prefetch_x(t + 1))
                attention_ic(0, t)
                if t - 1 >= 0:
                    queue_proj(t - 1)   # after att(0,t): tails t-1 flushed
                attention_ic(1, t)
                drain_urgent()
            while lazy_q:
                lazy_q.popleft()()
            # Final-chunk projection, pair-split: the p=0 partial matmuls
            # only need pair 0's attT (ready before the last tail), so they
            # run while the final tail's DVE/DMA chain drains; p=1 + the y
            # copy/DMA follow the tail.  PSUM for the partials is scrounged
            # from the sc/qk/av pools as attention releases them.
            tl = NTC - 1
            blocks = [
                (tb, nck)
                for tb in range(tl * (TC // 128), (tl + 1) * (TC // 128))
                for nck in range(C // 512)
            ]
            slots = []
            for i in range(min(4, len(blocks)) // 2):
                big = ps_sc.tile([128, 1024], FP32, tag="sc", name=f"pyb{i}")
                slots += [(big, 0), (big, 512)]
            for i in range(min(2, len(blocks) - len(slots))):
                tqk = ps_qk.tile([128, 512], FP32, tag="qk", name=f"pyq{i}")
                slots.append((tqk, 0))

            def proj_A(i):
                t_, off = slots[i]
                tb, nck = blocks[i]
                nc.tensor.matmul(
                    t_[:, off : off + 512],
                    attT[:, 0, 128 * tb : 128 * (tb + 1)],
                    wp_sb[:, 0, 512 * nck : 512 * (nck + 1)],
                    start=True,
                    stop=False,
                )

            def proj_B(i):
                t_, off = slots[i]
                tb, nck = blocks[i]
                nc.tensor.matmul(
                    t_[:, off : off + 512],
                    attT[:, 1, 128 * tb : 128 * (tb + 1)],
                    wp_sb[:, 1, 512 * nck : 512 * (nck + 1)],
                    start=False,
                    stop=True,
                )
                y_sb = ysb_pool.tile([128, 512], FP16, tag="ysb")
                nc.scalar.copy(out=y_sb, in_=t_[:, off : off + 512])
                eng = nc.sync if i % 2 == 0 else nc.scalar
                eng.dma_start(
                    y[128 * tb : 128 * (tb + 1), 512 * nck : 512 * (nck + 1)],
                    y_sb,
                )

            n_pre = len(slots)
            for i in range(n_pre):
                proj_A(i)
            emit_tail(*pending)
            for i in range(n_pre, len(blocks)):
                tav = ps_av.tile([128, 512], FP32, tag="av", name=f"pya{i}")
                slots.append((tav, 0))
                proj_A(i)
            for i in range(len(blocks)):
                proj_B(i)
    nc.compile()
    return nc


def make_in_maps(x, W_qkv, W_proj, n_cores=N_CORES):
    """Host-side sharding: per-core fp16 inputs."""
    Bv, T, C = x.shape
    groups = n_cores // Bv
    CD = C // groups
    in_maps = []
    xT_b = [np.ascontiguousarray(x[b].T).astype(np.float16) for b in range(Bv)]
    for m in range(n_cores):
        b, g = m // groups, m % groups
        cols = slice(CD * g, CD * (g + 1))
        wqkv_dev = np.ascontiguousarray(
            np.concatenate(
                [W_qkv[:, 0:C][:, cols], W_qkv[:, C : 2 * C][:, cols],
                 W_qkv[:, 2 * C : 3 * C][:, cols]],
                axis=1,
            ).astype(np.float16)
        )
        wp_dev = np.ascontiguousarray(W_proj[cols, :]).astype(np.float16)
        in_maps.append({"xT": xT_b[b], "wqkv": wqkv_dev, "wp": wp_dev})
    return in_maps


_cache = {}


def _get_nc():
    if "nc" not in _cache:
        _cache["nc"] = build_nc()
    return _cache["nc"]


def run(x, W_qkv, W_proj, trace=False):
    """Run on hardware; returns (y_full, BassKernelResults)."""
    from concourse.bass_utils import run_bass_kernel_spmd

    nc = _get_nc()
    in_maps = make_in_maps(x, W_qkv, W_proj)
    res = run_bass_kernel_spmd(
        nc, in_maps, core_ids=list(range(N_CORES)), trace=trace
    )
    Bv, T, C = x.shape
    groups = N_CORES // Bv
    y_full = np.zeros((Bv, T, C), dtype=np.float32)
    for m in range(N_CORES):
        y_full[m // groups] += res.results[m]["y"].astype(np.float32)
    return y_full, res


def kernel(x, W_qkv, W_proj):
    y, _ = run(
        np.asarray(x, dtype=np.float32),
        np.asarray(W_qkv, dtype=np.float32),
        np.asarray(W_proj, dtype=np.float32),
    )
    return y
